# revision 70
# baseline (speedup 1.0000x reference)
"""3-layer GAT (DGL-style GATConv) on 8 Trainium2 NeuronCores via Bass/Tile.

"Rounds" formulation (v3): dst-sharded, lane-per-node.
- Each core owns SH=N/8 dst nodes, sorted by in-degree (desc) and assigned to
  (block b, lane p): node q = b*128+p. Edges of node q occupy rounds r=0..deg-1.
  Rounds per block R_b = max in-degree in block b over all cores (SPMD).
- Per layer: table row (per node, bf16, 256 elems = 512B) = [f(128) | el(4) |
  pad]; AllGather to ftab [8*(SH+1), 256]. Gathers are split into two
  queue-parallel halves per group (SWDGE queues run concurrently on separate
  Q7 cpu pairs). Padding slots index a poisoned row (f=0, el=-1000 => a=0).
- e = el[src] + er[dst]: er[dst] is resident on the dst lane's partition
  (erexp, pre-expanded per round); a = exp(lrelu(e)) on the Scalar engine.
- Aggregation: gtf = a*f on DVE, then sum over rounds via PE identity-matmul
  accumulation in PSUM (z via tiny DVE reduce of a).
- Layer 2 factorized: aggregate alpha-weighted input h per head, project with
  W2 per head after transposing, relu(scale=1/z), max-pool.
- Head: AllReduce(max), fc + softmax (replicated). Output is invariant to the
  node permutation (global max-pool), so the sort is never undone.
"""
import numpy as np
import ml_dtypes

BF16 = ml_dtypes.bfloat16
P = 128
NC = 8
ROWW = 256   # table row elems (bf16) => 512B gather element
MAXG = 36    # max rounds per gather group
RC = 8       # rounds per gtf chunk (SBUF working-tile size)


def _ceil(a, b):
    return -(-a // b)


def _wrap16(seq):
    """dma_gather index layout: [128, n/16] int16, idx i at [i%16, i//16], replicated."""
    n = seq.shape[0]
    assert n % 16 == 0
    w = seq.reshape(n // 16, 16).T.astype(np.int16)  # [16, n/16]
    return np.tile(w, (8, 1))  # [128, n/16]


def _preprocess(src, dst, N, E):
    """Degree-sort nodes per core, build per-(lane,round) gather indices."""
    SH = N // NC
    NBLK = _ceil(SH, P)
    NTAB1 = SH + 1          # rows per core in the gathered table (incl poison)
    PAD_ROW = SH            # core 0's poison row

    deg = np.bincount(dst, minlength=N)
    orders = []
    pos = np.empty(N, np.int64)
    for c in range(NC):
        dl = deg[c * SH:(c + 1) * SH]
        order = np.argsort(-dl, kind="stable")
        orders.append(order)
        pos[c * SH + order] = np.arange(SH)
    R_b = np.zeros(NBLK, np.int64)
    for c in range(NC):
        ds = deg[c * SH:(c + 1) * SH][orders[c]]
        for b in range(NBLK):
            lo = b * P
            if lo < SH:
                R_b[b] = max(R_b[b], int(ds[lo:min(lo + P, SH)].max()))
    R_b = np.maximum(R_b, 1)

    cap = max(MAXG, int(R_b.max()))
    groups = []
    cur = []
    s = 0
    for b in range(NBLK):
        if cur and s + R_b[b] > cap:
            groups.append(cur)
            cur = []
            s = 0
        cur.append(b)
        s += R_b[b]
    if cur:
        groups.append(cur)
    grp_R = [int(sum(R_b[b] for b in g)) for g in groups]
    TOT_R = int(R_b.sum())

    src_row = (src // SH) * NTAB1 + pos[src]

    order_edges = np.argsort(dst, kind="stable")
    ss_row = src_row[order_edges]
    dd = dst[order_edges]
    q_of = pos[dd]
    core_of = dd // SH

    core_arrays = []
    for c in range(NC):
        m = core_of == c
        qs = q_of[m]
        rows = ss_row[m]
        o2 = np.argsort(qs, kind="stable")
        qs = qs[o2]
        rows = rows[o2]
        rr = np.arange(qs.shape[0]) - np.concatenate(
            ([0], np.cumsum(np.bincount(qs, minlength=SH))))[qs]
        idx2d = np.full((SH, int(R_b.max())), PAD_ROW, np.int64)
        idx2d[qs, rr] = rows
        fidx = np.zeros((P, TOT_R * 8), np.int16)
        off = 0
        for g, blks in enumerate(groups):
            seq = []
            for b in blks:
                lanes = np.arange(b * P, min((b + 1) * P, SH))
                blkidx = np.full((P, R_b[b]), PAD_ROW, np.int64)
                blkidx[:lanes.shape[0], :] = idx2d[lanes, :R_b[b]]
                seq.append(blkidx.T.reshape(-1))  # round-major, then lane
            seq = np.concatenate(seq)
            fidx[:, off * 8:(off + seq.shape[0] // P) * 8] = _wrap16(seq)
            off += seq.shape[0] // P
        core_arrays.append({"fidx": fidx})

    sched = {
        "SH": SH, "NBLK": NBLK, "R_b": [int(x) for x in R_b], "TOT_R": TOT_R,
        "groups": groups, "grp_R": grp_R, "NTAB1": NTAB1,
    }
    return sched, core_arrays


def _build_program(sched, FIN, phase=6):
    import concourse.bacc as bacc
    import concourse.mybir as mybir
    import concourse.tile as tile
    from concourse.masks import make_identity

    dt = mybir.dt
    SH, NBLK, R_b = sched["SH"], sched["NBLK"], sched["R_b"]
    TOT_R, groups, grp_R = sched["TOT_R"], sched["groups"], sched["grp_R"]
    NTAB1 = sched["NTAB1"]
    PBLK = _ceil(SH + 1, P)
    MAXGR = max(grp_R)
    AF = mybir.ActivationFunctionType
    OP = mybir.AluOpType

    nc = bacc.Bacc("TRN2", target_bir_lowering=False, debug=False, num_devices=NC,
                   num_swdge_queues=4)

    xT_in = nc.declare_dram_parameter("xT", [FIN, SH], dt.float32, isOutput=False)
    fidx_in = nc.declare_dram_parameter("fidx", [P, TOT_R * 8], dt.int16, isOutput=False)
    W0_in = nc.declare_dram_parameter("W0", [FIN, P], dt.float32, isOutput=False)
    W1_in = nc.declare_dram_parameter("W1", [P, P], dt.float32, isOutput=False)
    W2bf_in = nc.declare_dram_parameter("W2bf", [P, 4 * P], dt.bfloat16, isOutput=False)
    alel0_in = nc.declare_dram_parameter("alel0", [P, 8], dt.float32, isOutput=False)
    alel1_in = nc.declare_dram_parameter("alel1", [P, 8], dt.float32, isOutput=False)
    wal2_in = nc.declare_dram_parameter("wal2", [P, 8], dt.float32, isOutput=False)
    fcw_in = nc.declare_dram_parameter("fcw", [P, 4 * 8], dt.float32, isOutput=False)
    fcb_in = nc.declare_dram_parameter("fcb", [1, 8], dt.float32, isOutput=False)
    out_ext = nc.declare_dram_parameter("out", [1, 8], dt.float32, isOutput=True)
    dbg_ext = nc.declare_dram_parameter("dbg", [P, 512], dt.float32, isOutput=True)

    def dram(name, shape, dtype, shared=False):
        return nc.dram_tensor(name, shape, dtype,
                              addr_space="Shared" if shared else "Local")

    fsh = [dram(f"fsh{l}", [PBLK * P, ROWW], dt.bfloat16) for l in range(3)]
    ftab = [dram(f"ftab{l}", [NC * NTAB1, ROWW], dt.bfloat16, shared=True)
            for l in range(3)]
    pmax_in = dram("pmax_in", [P, 4], dt.float32)
    pmax_out = dram("pmax_out", [P, 4], dt.float32, shared=True)
    rg = [list(range(NC))]

    with tile.TileContext(nc) as tc:
        with (
            tc.tile_pool(name="const", bufs=1) as cp,
            tc.tile_pool(name="pers", bufs=1) as pers,
            tc.tile_pool(name="gath", bufs=4) as gp,
            tc.tile_pool(name="wk", bufs=6) as wk,
            tc.tile_pool(name="big", bufs=2) as bigp,
            tc.tile_pool(name="ep", bufs=4) as ep,
            tc.tile_pool(name="psum", bufs=5, space="PSUM") as pp,
            tc.tile_pool(name="psacc", bufs=2, space="PSUM") as pacc,
        ):
            f32, bf16 = dt.float32, dt.bfloat16

            def load_const(name, src_ap, shape, dtype):
                t = cp.tile(shape, dtype, tag=name)
                nc.sync.dma_start(out=t[:], in_=src_ap)
                return t

            fidx_sb = load_const("fidx", fidx_in[:], [P, TOT_R * 8], dt.int16)
            W0_sb = load_const("W0", W0_in[:], [FIN, P], f32)
            W1_sb = load_const("W1", W1_in[:], [P, P], f32)
            W2bf_sb = load_const("W2bf", W2bf_in[:], [P, 4 * P], bf16)
            alel_sb = [load_const("alel0", alel0_in[:], [P, 8], f32),
                       load_const("alel1", alel1_in[:], [P, 8], f32), None]
            wal2_sb = load_const("wal2", wal2_in[:], [P, 8], f32)
            fcw_sb = load_const("fcw", fcw_in[:], [P, 4 * 8], f32)
            fcb_sb = load_const("fcb", fcb_in[:], [1, 8], f32)
            ident = cp.tile([P, P], f32, tag="identf")
            make_identity(nc, ident[:])
            identbf = cp.tile([P, P], bf16, tag="identbf")
            nc.vector.tensor_copy(out=identbf[:], in_=ident[:])

            ersb = pers.tile([P, NBLK * 4], f32, tag="ersb")
            erexp = pers.tile([P, TOT_R * 4], f32, tag="erexp")

            def stage_prep(l, hT_sb):
                """Project, compute per-node el/er, build+poison table, AllGather."""
                W_sb = [W0_sb, W1_sb, None][l]
                fbf = pers.tile([P, PBLK * ROWW], bf16, tag="fbf")
                if l < 2:
                    fT = pers.tile([P, SH], f32, tag="fT")
                    nn = 0
                    while nn < SH:
                        w = min(512, SH - nn)
                        ftp = pp.tile([P, 512], f32, tag="pp")
                        nc.tensor.matmul(out=ftp[:, :w], lhsT=W_sb[:],
                                         rhs=hT_sb[:, nn:nn + w],
                                         start=True, stop=True)
                        nc.vector.tensor_copy(out=fT[:, nn:nn + w], in_=ftp[:, :w])
                        nn += w
                    srcT = fT
                else:
                    srcT = hT_sb
                rhs_er = wal2_sb[:] if l == 2 else alel_sb[l][:]
                # poison the trailing table block first; the loop overwrites
                # the valid lanes
                lb = PBLK - 1
                nc.gpsimd.memset(fbf[:, lb * ROWW:lb * ROWW + 132], 0.0)
                nc.gpsimd.memset(fbf[:, lb * ROWW + 128:lb * ROWW + 132], -1000.0)
                for b in range(NBLK):
                    nv = min(P, SH - b * P)
                    co = b * ROWW
                    erp = pp.tile([P, 8], f32, tag="pp")
                    nc.tensor.matmul(out=erp[:nv, :],
                                     lhsT=srcT[:, b * P:b * P + nv],
                                     rhs=rhs_er, start=True, stop=True)
                    nc.vector.tensor_copy(out=fbf[:nv, co + 128:co + 132],
                                          in_=erp[:nv, 0:4])
                    nc.vector.tensor_copy(out=ersb[:nv, b * 4:(b + 1) * 4],
                                          in_=erp[:nv, 4:8])
                    trp = pp.tile([P, P], f32, tag="pp")
                    nc.tensor.transpose(out=trp[:nv, :],
                                        in_=srcT[:, b * P:b * P + nv],
                                        identity=ident[:])
                    nc.vector.tensor_copy(out=fbf[:nv, co:co + P], in_=trp[:nv, :])
                nc.sync.dma_start(
                    out=fsh[l].rearrange("(b p) f -> p b f", p=P)[:, :, 0:ROWW],
                    in_=fbf[:].rearrange("p (b w) -> p b w", w=ROWW))
                nc.gpsimd.collective_compute(
                    "AllGather", OP.bypass,
                    ins=[fsh[l][:NTAB1, :]], outs=[ftab[l][:]], replica_groups=rg)
                # expand er per round (per-block broadcast)
                roff = 0
                for b in range(NBLK):
                    R = R_b[b]
                    nc.vector.tensor_copy(
                        out=erexp[:, roff * 4:(roff + R) * 4]
                            .rearrange("p (r h) -> p r h", h=4),
                        in_=ersb[:, b * 4:(b + 1) * 4].unsqueeze(1)
                            .to_broadcast([P, R, 4]))
                    roff += R

            def _epilogue01(b, ps, zi, hT_next):
                nv = min(P, SH - b * P)
                hdiv = ep.tile([P, P], f32, tag="hdiv")
                nc.vector.tensor_tensor(
                    out=hdiv[:].rearrange("p (h c) -> p h c", c=32),
                    in0=ps[:, 0:128].rearrange("p (h c) -> p h c", c=32),
                    in1=zi[:].unsqueeze(-1).to_broadcast([P, 4, 32]),
                    op=OP.mult)
                hre = ep.tile([P, P], f32, tag="hre")
                nc.scalar.activation(out=hre[:], in_=hdiv[:], func=AF.Relu)
                trp = pp.tile([P, P], f32, tag="pp")
                nc.tensor.transpose(out=trp[:], in_=hre[:], identity=ident[:])
                nc.vector.tensor_copy(out=hT_next[:, b * P:b * P + nv],
                                      in_=trp[:, :nv])

            def _epilogue2(b, ps, zi, acc_max):
                nv = min(P, SH - b * P)
                agg = ep.tile([P, 512], bf16, tag="agg")
                nc.vector.tensor_copy(out=agg[:], in_=ps[:])
                o2 = pp.tile([P, 512], f32, tag="pp")
                for h in range(4):
                    trp = pp.tile([P, P], bf16, tag="pp")
                    nc.tensor.transpose(out=trp[:], in_=agg[:, h * P:(h + 1) * P],
                                        identity=identbf[:])
                    aggT = ep.tile([P, P], bf16, tag="aggT")
                    nc.vector.tensor_copy(out=aggT[:], in_=trp[:])
                    nc.tensor.matmul(out=o2[:, h * P:(h + 1) * P], lhsT=aggT[:],
                                     rhs=W2bf_sb[:, h * P:(h + 1) * P],
                                     start=True, stop=True)
                o2r = ep.tile([P, 512], f32, tag="o2r")
                for h in range(4):
                    nc.scalar.activation(out=o2r[:, h * P:(h + 1) * P],
                                         in_=o2[:, h * P:(h + 1) * P],
                                         func=AF.Relu, scale=zi[:, h:h + 1])
                nc.vector.tensor_tensor(out=acc_max[:nv, :], in0=acc_max[:nv, :],
                                        in1=o2r[:nv, :], op=OP.max)

            def layer_main(l, hT_next, acc_max):
                fw = 512 if l == 2 else P
                off_all = 0
                pending = None

                def flush():
                    nonlocal pending
                    if pending is not None:
                        pb, pps, pab, pR = pending
                        z4 = wk.tile([P, 4], f32, tag="z4")
                        nc.vector.reduce_sum(
                            out=z4[:],
                            in_=pab.rearrange("p (r h) -> p h r", h=4),
                            axis=mybir.AxisListType.X)
                        zi = wk.tile([P, 4], f32, tag="zi")
                        nc.vector.tensor_scalar(out=zi[:], in0=z4[:],
                                                scalar1=1e-30, scalar2=None,
                                                op0=OP.add)
                        nc.vector.reciprocal(out=zi[:], in_=zi[:])
                        if l < 2:
                            _epilogue01(pb, pps, zi, hT_next)
                        else:
                            _epilogue2(pb, pps, zi, acc_max)
                        pending = None

                for g, blks in enumerate(groups):
                    gR = grp_R[g]
                    o8 = off_all * 8
                    # emit the previous group's last epilogue BEFORE the gather
                    # instructions: the sync pass orders ops on a shared
                    # counting semaphore, so anything emitted after a gather
                    # transitively waits for its ~18us engine slice
                    flush()
                    fg = gp.tile([P, MAXGR, ROWW], bf16, tag="fg")
                    # queue-parallel gather splits (concurrent desc-gen on
                    # separate Q7 cpu pairs); short slices also shrink the
                    # sync-pass convoy stalls on unrelated ops
                    if gR >= 16:
                        cuts = [0, gR // 4, gR // 2, (3 * gR) // 4, gR]
                    elif gR >= 12:
                        cuts = [0, gR // 3, (2 * gR) // 3, gR]
                    elif gR >= 8:
                        cuts = [0, gR // 2, gR]
                    else:
                        cuts = [0, gR]
                    for j in range(len(cuts) - 1):
                        r0, r1 = cuts[j], cuts[j + 1]
                        nc.gpsimd.dma_gather(
                            out_ap=fg[:, r0:r1, :], in_ap=ftab[l][:],
                            idxs_ap=fidx_sb[:, o8 + r0 * 8:o8 + r1 * 8],
                            num_idxs=(r1 - r0) * P, num_idxs_reg=(r1 - r0) * P,
                            elem_size=ROWW, single_packet=False,
                            queue_num=(3 * g + j) % 4)
                    # group-wide attention scores
                    e4g = wk.tile([P, MAXGR * 4], f32, tag="e4g")
                    nc.vector.tensor_tensor(
                        out=e4g[:, :gR * 4].rearrange("p (r h) -> p r h", h=4),
                        in0=fg[:, :gR, 128:132],
                        in1=erexp[:, off_all * 4:(off_all + gR) * 4]
                            .rearrange("p (r h) -> p r h", h=4),
                        op=OP.add)
                    a1 = wk.tile([P, MAXGR * 4], bf16, tag="a1")
                    a2 = wk.tile([P, MAXGR * 4], bf16, tag="a2")
                    nc.scalar.activation(out=a1[:, :gR * 4], in_=e4g[:, :gR * 4],
                                         func=AF.Exp)
                    nc.scalar.activation(out=a2[:, :gR * 4], in_=e4g[:, :gR * 4],
                                         func=AF.Exp, scale=0.2)
                    ag = wk.tile([P, MAXGR * 4], bf16, tag="ag")
                    nc.vector.tensor_tensor(out=ag[:, :gR * 4], in0=a1[:, :gR * 4],
                                            in1=a2[:, :gR * 4], op=OP.max)
                    roff = 0
                    for b in blks:
                        R = R_b[b]
                        ab = ag[:, roff * 4:(roff + R) * 4]
                        ps = pacc.tile([P, 512], f32, tag="ps")
                        for r0 in range(0, R, RC):
                            rc = min(RC, R - r0)
                            fgb = fg[:, roff + r0:roff + r0 + rc, :]
                            abc = ag[:, (roff + r0) * 4:(roff + r0 + rc) * 4]
                            gtf = bigp.tile([P, 512 * RC], bf16, tag="gtf")
                            gv = gtf[:, :fw * rc]
                            if l < 2:
                                nc.vector.tensor_tensor(
                                    out=gv.rearrange("p (r h c) -> p r h c",
                                                     h=4, c=32),
                                    in0=fgb[:, :, 0:128]
                                        .rearrange("p r (h c) -> p r h c", c=32),
                                    in1=abc.rearrange("p (r h) -> p r h", h=4)
                                        .unsqueeze(-1).to_broadcast([P, rc, 4, 32]),
                                    op=OP.mult)
                            else:
                                nc.vector.tensor_tensor(
                                    out=gv.rearrange("p (r h c) -> p r h c",
                                                     h=4, c=128),
                                    in0=fgb[:, :, 0:128].unsqueeze(2)
                                        .to_broadcast([P, rc, 4, 128]),
                                    in1=abc.rearrange("p (r h) -> p r h", h=4)
                                        .unsqueeze(-1).to_broadcast([P, rc, 4, 128]),
                                    op=OP.mult)
                            for r in range(rc):
                                nc.tensor.matmul(
                                    out=ps[:, :fw], lhsT=identbf[:],
                                    rhs=gv.rearrange("p (r f) -> p r f",
                                                     f=fw)[:, r, :],
                                    start=(r0 == 0 and r == 0),
                                    stop=(r0 + rc == R and r == rc - 1))
                        flush()
                        pending = (b, ps, ab, R)
                        roff += R
                    off_all += gR
                flush()

            # ================= debug helpers =================
            def dump_dbg(ap_f32_cols):
                dt_ = pers.tile([P, 512], f32, tag="dbgt")
                nc.gpsimd.memset(dt_[:], 0.0)
                for ap, c0, w in ap_f32_cols:
                    nc.vector.tensor_copy(out=dt_[:, c0:c0 + w], in_=ap)
                nc.sync.dma_start(out=dbg_ext[:], in_=dt_[:])
                dd = ep.tile([1, 8], f32, tag="ot")
                nc.gpsimd.memset(dd[:], 0.5)
                nc.sync.dma_start(out=out_ext[:], in_=dd[:])

            def dump_tab(l):
                tb = ep.tile([P, ROWW], bf16, tag="dump1")
                nc.sync.dma_start(out=tb[:], in_=ftab[l][0:P, :])
                cols = [(tb[:, :P], 0, P), (tb[:, 128:132], 128, 4)]
                dump_dbg(cols)

            # ================= run the network =================
            hT0 = pers.tile([P, SH], f32, tag="hTa")
            nc.sync.dma_start(out=hT0[:FIN, :], in_=xT_in[:])
            stage_prep(0, hT0)
            if phase == 0:
                dump_tab(0)
            if phase >= 1:
                hT1 = pers.tile([P, SH], f32, tag="hTb")
                layer_main(0, hT1, None)
                if phase == 1:
                    dump_dbg([(hT1[:, 0:min(512, SH)], 0, min(512, SH))])
            if phase >= 2:
                stage_prep(1, hT1)
                if phase == 2:
                    dump_tab(1)
            if phase >= 3:
                hT2 = pers.tile([P, SH], f32, tag="hTa")
                layer_main(1, hT2, None)
                if phase == 3:
                    dump_dbg([(hT2[:, 0:min(512, SH)], 0, min(512, SH))])
            if phase >= 4:
                stage_prep(2, hT2)
                if phase == 4:
                    dump_tab(2)
            if phase >= 5:
                acc_max = pers.tile([P, 512], f32, tag="accmax")
                nc.gpsimd.memset(acc_max[:], 0.0)
                layer_main(2, None, acc_max)
                if phase == 5:
                    dump_dbg([(acc_max[:], 0, 512)])

            def head():
                pooledT = ep.tile([P, 4], f32, tag="pooledT")
                for j in range(4):
                    trp = pp.tile([P, P], f32, tag="pp")
                    nc.tensor.transpose(out=trp[:], in_=acc_max[:, j * P:(j + 1) * P],
                                        identity=ident[:])
                    nc.vector.reduce_max(out=pooledT[:, j:j + 1], in_=trp[:],
                                         axis=mybir.AxisListType.X)
                nc.sync.dma_start(out=pmax_in[:], in_=pooledT[:])
                nc.gpsimd.collective_compute(
                    "AllReduce", OP.max,
                    ins=[pmax_in[:]], outs=[pmax_out[:]], replica_groups=rg)
                pm = ep.tile([P, 4], f32, tag="pm")
                nc.sync.dma_start(out=pm[:], in_=pmax_out[:])
                fcp = pp.tile([1, 8], f32, tag="pp")
                for j in range(4):
                    nc.tensor.matmul(out=fcp[:], lhsT=pm[:, j:j + 1],
                                     rhs=fcw_sb[:, j * 8:(j + 1) * 8],
                                     start=(j == 0), stop=(j == 3))
                lg = ep.tile([1, 8], f32, tag="lg")
                nc.vector.tensor_tensor(out=lg[:], in0=fcp[:], in1=fcb_sb[:], op=OP.add)
                mx = ep.tile([1, 1], f32, tag="mx")
                nc.vector.reduce_max(out=mx[:], in_=lg[:], axis=mybir.AxisListType.X)
                nc.vector.tensor_tensor(out=lg[:], in0=lg[:],
                                        in1=mx[:].to_broadcast([1, 8]), op=OP.subtract)
                ex = ep.tile([1, 8], f32, tag="ex")
                nc.scalar.activation(out=ex[:], in_=lg[:], func=AF.Exp)
                sm = ep.tile([1, 1], f32, tag="sm")
                nc.vector.reduce_sum(out=sm[:], in_=ex[:], axis=mybir.AxisListType.X)
                nc.vector.reciprocal(out=sm[:], in_=sm[:])
                ot = ep.tile([1, 8], f32, tag="ot")
                nc.vector.tensor_tensor(out=ot[:], in0=ex[:],
                                        in1=sm[:].to_broadcast([1, 8]), op=OP.mult)
                nc.sync.dma_start(out=out_ext[:], in_=ot[:])

            if phase >= 6:
                head()

    nc.finalize()
    return nc


def _host_consts(W0, al0, ar0, W1, al1, ar1, W2, al2, ar2, fc_w, fc_b):
    def foldmat(v):
        hh, cc = v.shape
        m = np.zeros((hh * cc, hh), np.float32)
        for h in range(hh):
            m[h * cc:(h + 1) * cc, h] = v[h]
        return m

    def alel(al, ar):
        return np.concatenate([foldmat(al), foldmat(ar)], axis=1)  # [128, 8]

    wal2 = np.zeros((P, 8), np.float32)
    wal2[:, 0:4] = (W2.astype(np.float64) @ foldmat(al2).astype(np.float64)).astype(np.float32)
    wal2[:, 4:8] = (W2.astype(np.float64) @ foldmat(ar2).astype(np.float64)).astype(np.float32)
    fcw = np.ascontiguousarray(
        fc_w.reshape(4, P, 8).transpose(1, 0, 2).reshape(P, 32)).astype(np.float32)
    return {
        "W0": np.ascontiguousarray(W0).astype(np.float32),
        "W1": np.ascontiguousarray(W1).astype(np.float32),
        "W2bf": np.ascontiguousarray(W2).astype(BF16),
        "alel0": alel(al0, ar0), "alel1": alel(al1, ar1),
        "wal2": wal2,
        "fcw": fcw, "fcb": fc_b.reshape(1, 8).astype(np.float32),
    }


_PROG_CACHE = {}


def run_gat(inputs, src, dst, W0, al0, ar0, W1, al1, ar1, W2, al2, ar2, fc_w, fc_b,
            trace=False):
    from concourse.bass_utils import run_bass_kernel_spmd
    inputs = np.asarray(inputs)
    N, FIN = inputs.shape
    E = np.asarray(src).shape[0]
    sched, core_arrays = _preprocess(np.asarray(src).astype(np.int64),
                                     np.asarray(dst).astype(np.int64), N, E)
    import os
    phase = int(os.environ.get("GAT_PHASE", "6"))
    key = (N, E, FIN, tuple(sched["R_b"]), phase)
    if key not in _PROG_CACHE:
        _PROG_CACHE[key] = _build_program(sched, FIN, phase)
    nc = _PROG_CACHE[key]
    consts = _host_consts(np.asarray(W0), np.asarray(al0), np.asarray(ar0),
                          np.asarray(W1), np.asarray(al1), np.asarray(ar1),
                          np.asarray(W2), np.asarray(al2), np.asarray(ar2),
                          np.asarray(fc_w), np.asarray(fc_b))
    SH = sched["SH"]
    deg = np.bincount(np.asarray(dst), minlength=N)
    in_maps = []
    for c in range(NC):
        m = dict(consts)
        m.update(core_arrays[c])
        order = np.argsort(-deg[c * SH:(c + 1) * SH], kind="stable")
        m["xT"] = np.ascontiguousarray(
            inputs[c * SH:(c + 1) * SH, :][order, :].T).astype(np.float32)
        in_maps.append(m)
    res = run_bass_kernel_spmd(nc, in_maps, list(range(NC)), trace=trace)
    out = np.asarray(res.results[0]["out"])
    run_gat.last_dbg = np.asarray(res.results[0].get("dbg")) if "dbg" in res.results[0] else None
    return out, res


def kernel(**inputs):
    out, _ = run_gat(**inputs)
    return out
